# revision 12
# baseline (speedup 1.0000x reference)
"""Batched NMS (torchvision semantics) on 8 Trainium2 cores.

Strategy (per-class expert-parallel, exact-match to the jax reference):
  host   : route boxes by class into a padded per-class slot layout
           (80 classes x 192 slots, 10 classes per core) -- routing only.
  device : P1 within-class priority ranking (score desc, orig idx asc)
           P2 reorder slots into priority order (indirect scatter)
           P3 per-class IoU matrices on offset boxes (f32, reference chain)
           P4 greedy suppression sweep, vectorized across classes
           AllReduce keep mask
           P5 global rank of scores-after-nms == argsort(-scores_after)
           P6 scatter final rows of (boxes, score) and indices
"""
import os
import numpy as np

N = 8192
C = 80
S = 192            # per-class slot capacity (overflow prob ~1e-11)
NCORES = 8
CPC = C // NCORES  # classes per core = 10
NT = 2 * CPC       # row tiles per core (96 rows each) = 20
NSLOT = CPC * S    # slots per core = 1920
NCH = 8            # row chunks per core for final ranking
T45 = float(np.float32(0.45))

DEBUG = bool(int(os.environ.get("DEBUG_NMS", "0")))

_cache = {}


def _build():
    import concourse.bass as bass
    import concourse.mybir as mybir
    import concourse.tile as tile
    import concourse.bacc as bacc

    F = mybir.dt.float32
    BF = mybir.dt.bfloat16
    I32 = mybir.dt.int32
    Op = mybir.AluOpType
    AX = mybir.AxisListType
    IOffs = bass.IndirectOffsetOnAxis

    nc = bacc.Bacc("TRN2", target_bir_lowering=False, debug=False,
                   num_devices=NCORES)

    boxes_d = nc.dram_tensor("boxes", [N, 4], F, kind="ExternalInput")
    scores_d = nc.dram_tensor("scores", [N], F, kind="ExternalInput")
    sboxes_d = nc.dram_tensor("slot_boxes", [NSLOT, 4], F, kind="ExternalInput")
    sscore_d = nc.dram_tensor("slot_scores", [NSLOT], F, kind="ExternalInput")
    sorig_d = nc.dram_tensor("slot_origf", [NSLOT], F, kind="ExternalInput")
    clstq_d = nc.dram_tensor("cls_tq", [NT, 4], F, kind="ExternalInput")
    crows_d = nc.dram_tensor("chunk_rows", [128, NCH], I32, kind="ExternalInput")
    ob_d = nc.dram_tensor("out_boxes", [N, 5], F, kind="ExternalOutput")
    inds_d = nc.dram_tensor("out_inds", [N], I32, kind="ExternalOutput")
    dbg = {}
    if DEBUG:
        dbg["wrank"] = nc.dram_tensor("dbg_wrank", [96, NT], F, kind="ExternalOutput")
        dbg["prec"] = nc.dram_tensor("dbg_prec", [NSLOT, 8], F, kind="ExternalOutput")
        dbg["alive"] = nc.dram_tensor("dbg_alive", [CPC, S], F, kind="ExternalOutput")
        dbg["keep"] = nc.dram_tensor("dbg_keep", [N], F, kind="ExternalOutput")
        dbg["rank"] = nc.dram_tensor("dbg_rank", [128, NCH], F, kind="ExternalOutput")

    with tile.TileContext(nc) as tc:
        with (
            tc.tile_pool(name="const", bufs=1) as cp,
            tc.tile_pool(name="work", bufs=3) as wp,
            tc.tile_pool(name="psum", bufs=2, space="PSUM") as pp,
            tc.tile_pool(name="dram", bufs=1, space="DRAM") as dp,
        ):
            # ---------- constants ----------
            iota192i = cp.tile([96, S], I32)
            nc.gpsimd.iota(iota192i[:], pattern=[[1, S]], base=0, channel_multiplier=0)
            iota192 = cp.tile([96, S], F)
            nc.vector.tensor_copy(iota192[:], iota192i[:])
            prow0i = cp.tile([96, 1], I32)
            nc.gpsimd.iota(prow0i[:], pattern=[[0, 1]], base=0, channel_multiplier=1)
            prow0 = cp.tile([96, 1], F)
            nc.vector.tensor_copy(prow0[:], prow0i[:])
            prow96i = cp.tile([96, 1], I32)
            nc.gpsimd.iota(prow96i[:], pattern=[[0, 1]], base=96, channel_multiplier=1)
            prow96 = cp.tile([96, 1], F)
            nc.vector.tensor_copy(prow96[:], prow96i[:])
            ltm = []   # ltm[h][p,k] = (k < 96h+p)
            upm = []   # upm[h][p,k] = (k > 96h+p)
            for h, pr in enumerate((prow0, prow96)):
                lt = cp.tile([96, S], F, tag=f"ltm{h}")
                nc.vector.tensor_scalar(out=lt[:], in0=iota192[:], scalar1=pr[:],
                                        scalar2=None, op0=Op.is_lt)
                ltm.append(lt)
                up = cp.tile([96, S], F, tag=f"upm{h}")
                nc.vector.tensor_scalar(out=up[:], in0=iota192[:], scalar1=pr[:],
                                        scalar2=None, op0=Op.is_gt)
                upm.append(up)
            # classbase[p, t] = S * (t // 2)
            cbase = cp.tile([96, NT], F)
            cbi = cp.tile([96, NT], I32)
            nc.gpsimd.iota(cbi[:], pattern=[[1, CPC], [0, 2]], base=0,
                           channel_multiplier=0)
            nc.vector.tensor_scalar(out=cbase[:], in0=cbi[:], scalar1=float(S),
                                    scalar2=None, op0=Op.mult)
            io1024i = cp.tile([128, 1024], I32)
            nc.gpsimd.iota(io1024i[:], pattern=[[1, 1024]], base=0, channel_multiplier=0)
            io1024 = cp.tile([128, 1024], F)
            nc.vector.tensor_copy(io1024[:], io1024i[:])

            # ---------- P0: maxc + 1 ----------
            ball = wp.tile([128, 256], F, tag="ball")
            nc.gpsimd.dma_start(ball[:], boxes_d[:].rearrange("(p f) c -> p (f c)", p=128))
            rmax = wp.tile([128, 1], F, tag="rmax")
            nc.vector.tensor_reduce(out=rmax[:], in_=ball[:], axis=AX.X, op=Op.max)
            mx_dram = dp.tile([128, 1], F)
            nc.gpsimd.dma_start(mx_dram[:], rmax[:])
            mxrow = wp.tile([1, 128], F, tag="mxrow")
            nc.gpsimd.dma_start(mxrow[:], mx_dram[:].rearrange("n o -> o n"))
            maxp1 = wp.tile([1, 1], F, tag="maxp1")
            nc.vector.tensor_reduce(out=maxp1[:], in_=mxrow[:], axis=AX.X, op=Op.max)
            nc.vector.tensor_scalar(out=maxp1[:], in0=maxp1[:], scalar1=1.0,
                                    scalar2=None, op0=Op.add)
            mp1b = cp.tile([96, 1], F)
            nc.gpsimd.partition_broadcast(mp1b[:], maxp1[:])

            # ---------- P1: within-class priority ranks (slot order) ----------
            _p1cm = tc.tile_pool(name="p1big", bufs=1)
            p1p = _p1cm.__enter__()
            s_colb = p1p.tile([96, NSLOT], F, tag="scolb")
            nc.gpsimd.dma_start(s_colb[0:1, :], sscore_d[:].rearrange("(o n) -> o n", o=1))
            nc.gpsimd.partition_broadcast(s_colb[:], s_colb[0:1, :])
            s_rowp = wp.tile([96, NT], F, tag="srowp")   # slot scores, (p, t)
            nc.gpsimd.dma_start(s_rowp[:], sscore_d[:].rearrange("(t p) -> p t", p=96))
            wrank = wp.tile([96, NT], F, tag="wrank")
            for t in range(NT):
                cs = S * (t // 2)
                eqlt = wp.tile([96, S], F, tag="eqlt")
                nc.vector.scalar_tensor_tensor(
                    out=eqlt[:], in0=s_colb[:, cs:cs + S],
                    scalar=s_rowp[:, t:t + 1], in1=ltm[t % 2][:],
                    op0=Op.is_equal, op1=Op.mult)
                junk = wp.tile([96, S], F, tag="junkp1")
                nc.vector.scalar_tensor_tensor(
                    out=junk[:], in0=s_colb[:, cs:cs + S],
                    scalar=s_rowp[:, t:t + 1], in1=eqlt[:],
                    op0=Op.is_gt, op1=Op.add, accum_out=wrank[:, t:t + 1])
            slotf = wp.tile([96, NT], F, tag="slotf")
            nc.vector.tensor_tensor(out=slotf[:], in0=wrank[:], in1=cbase[:], op=Op.add)
            sloti = wp.tile([96, NT], I32, tag="sloti")
            nc.vector.tensor_copy(sloti[:], slotf[:])
            if DEBUG:
                nc.gpsimd.dma_start(dbg["wrank"][:], wrank[:])
            _p1cm.__exit__(None, None, None)

            # ---------- P2: build records and scatter into priority order ----------
            sbx = wp.tile([96, NT, 4], F, tag="sbx")
            nc.gpsimd.dma_start(sbx[:], sboxes_d[:].rearrange("(t p) c -> p t c", p=96))
            clsm = wp.tile([96, NT, 4], F, tag="clsm")
            crow = wp.tile([1, 4 * NT], F, tag="crow")
            nc.gpsimd.dma_start(crow[:], clstq_d[:].rearrange("t c -> (t c)").rearrange("(o n) -> o n", o=1))
            nc.gpsimd.partition_broadcast(clsm[:], crow[:])
            rec = wp.tile([96, NT, 8], F, tag="rec")
            # offset boxes: rec[:, :, 0:4] = cls*(maxc+1) + box
            nc.vector.scalar_tensor_tensor(
                out=rec[:, :, 0:4], in0=clsm[:], scalar=mp1b[:], in1=sbx[:],
                op0=Op.mult, op1=Op.add)
            # area = (x2-x1)*(y2-y1) on offset boxes -> rec[:, :, 4]
            w_ = wp.tile([96, NT], F, tag="w_")
            h_ = wp.tile([96, NT], F, tag="h_")
            nc.vector.tensor_tensor(out=w_[:], in0=rec[:, :, 2], in1=rec[:, :, 0], op=Op.subtract)
            nc.vector.tensor_tensor(out=h_[:], in0=rec[:, :, 3], in1=rec[:, :, 1], op=Op.subtract)
            nc.vector.tensor_tensor(out=rec[:, :, 4], in0=w_[:], in1=h_[:], op=Op.mult)
            # score, origf direct DMA into record slices
            nc.gpsimd.dma_start(rec[:, :, 5], sscore_d[:].rearrange("(t p) -> p t", p=96))
            nc.gpsimd.dma_start(rec[:, :, 6], sorig_d[:].rearrange("(t p) -> p t", p=96))
            nc.vector.memset(rec[:, :, 7], 0.0)
            p_rec = dp.tile([NSLOT, 8], F)
            for t in range(NT):
                nc.gpsimd.indirect_dma_start(
                    out=p_rec[:], out_offset=IOffs(ap=sloti[:, t:t + 1], axis=0),
                    in_=rec[:, t, :], in_offset=None)
            if DEBUG:
                nc.gpsimd.dma_start(dbg["prec"][:], p_rec[:])

            # ---------- P3: per-class IoU tiles ----------
            _swcm = tc.tile_pool(name="swbig", bufs=1)
            swp_ = _swcm.__enter__()
            _colcm = tc.tile_pool(name="colbig", bufs=1)
            colp = _colcm.__enter__()
            colb = {}
            for name, f in (("x1", 0), ("y1", 1), ("x2", 2), ("y2", 3)):
                cb = colp.tile([96, NSLOT], F, tag=f"col{name}")
                nc.gpsimd.dma_start(cb[0:1, :], p_rec[:, f:f + 1].rearrange("n o -> o n"))
                nc.gpsimd.partition_broadcast(cb[:], cb[0:1, :])
                colb[name] = cb
            ones_r = colp.tile([1, NSLOT], F)
            nc.vector.memset(ones_r[:], 1.0)
            lhsT = colp.tile([2, NSLOT], F)
            nc.gpsimd.dma_start(lhsT[0:1, :], p_rec[:, 4:5].rearrange("n o -> o n"))
            nc.gpsimd.dma_start(lhsT[1:2, :], ones_r[:])
            rhs = colp.tile([2, NSLOT], F)
            nc.gpsimd.dma_start(rhs[0:1, :], ones_r[:])
            nc.gpsimd.dma_start(rhs[1:2, :], p_rec[:, 4:5].rearrange("n o -> o n"))
            rowrec = wp.tile([96, NT, 8], F, tag="rowrec")
            nc.gpsimd.dma_start(rowrec[:], p_rec[:].rearrange("(t p) f -> p t f", p=96))

            sw = swp_.tile([CPC, S * S], BF, tag="sweep")
            for t in range(NT):
                cs = S * (t // 2)
                h = t % 2
                c = t // 2
                t1 = wp.tile([96, S], F, tag="t1")
                nc.vector.tensor_scalar(out=t1[:], in0=colb["x2"][:, cs:cs + S],
                                        scalar1=rowrec[:, t, 2:3], scalar2=None, op0=Op.min)
                t2 = wp.tile([96, S], F, tag="t2")
                nc.vector.scalar_tensor_tensor(out=t2[:], in0=colb["x1"][:, cs:cs + S],
                                               scalar=rowrec[:, t, 0:1], in1=t1[:],
                                               op0=Op.max, op1=Op.subtract)
                wt = wp.tile([96, S], F, tag="wt")
                nc.scalar.activation(out=wt[:], in_=t2[:],
                                     func=mybir.ActivationFunctionType.Relu, scale=-1.0)
                t3 = wp.tile([96, S], F, tag="t3")
                nc.vector.tensor_scalar(out=t3[:], in0=colb["y2"][:, cs:cs + S],
                                        scalar1=rowrec[:, t, 3:4], scalar2=None, op0=Op.min)
                t4 = wp.tile([96, S], F, tag="t4")
                nc.vector.scalar_tensor_tensor(out=t4[:], in0=colb["y1"][:, cs:cs + S],
                                               scalar=rowrec[:, t, 1:2], in1=t3[:],
                                               op0=Op.max, op1=Op.subtract)
                ht = wp.tile([96, S], F, tag="ht")
                nc.scalar.activation(out=ht[:], in_=t4[:],
                                     func=mybir.ActivationFunctionType.Relu, scale=-1.0)
                inter = wp.tile([96, S], F, tag="inter")
                nc.vector.tensor_tensor(out=inter[:], in0=wt[:], in1=ht[:], op=Op.mult)
                asum = pp.tile([96, S], F, tag="asum")
                nc.tensor.matmul(asum[:], lhsT[:, 96 * t:96 * (t + 1)],
                                 rhs[:, cs:cs + S], start=True, stop=True)
                u_ = wp.tile([96, S], F, tag="u_")
                nc.vector.tensor_tensor(out=u_[:], in0=asum[:], in1=inter[:], op=Op.subtract)
                dec = wp.tile([96, S], F, tag="dec")
                nc.vector.scalar_tensor_tensor(out=dec[:], in0=u_[:], scalar=T45,
                                               in1=inter[:], op0=Op.mult, op1=Op.is_lt)
                decb = wp.tile([96, S], BF, tag="decb")
                nc.vector.tensor_tensor(out=decb[:], in0=dec[:], in1=upm[h][:], op=Op.mult)
                # rearrange [96 rows, S cols] -> sw[c, (96h+p)*S + k]
                dst = sw[c:c + 1, S * 96 * h: S * 96 * h + 96 * S]
                nc.gpsimd.dma_start(dst.rearrange("o (p k) -> o p k", k=S), decb[:])

            _colcm.__exit__(None, None, None)

            # ---------- P4: greedy sweep across classes ----------
            alive = swp_.tile([CPC, S], BF, tag="alive")
            nc.vector.memset(alive[:], 1.0)
            for t in range(S):
                nc.vector.scalar_tensor_tensor(
                    out=alive[:], in0=sw[:, S * t:S * (t + 1)],
                    scalar=alive[:, t:t + 1], in1=alive[:],
                    op0=Op.mult, op1=Op.is_lt)
            alf = wp.tile([CPC, S], F, tag="alf")
            nc.vector.tensor_copy(alf[:], alive[:])
            _swcm.__exit__(None, None, None)
            if DEBUG:
                nc.gpsimd.dma_start(dbg["alive"][:], alf[:])
            kflat = dp.tile([CPC, S], F)
            nc.gpsimd.dma_start(kflat[:], alf[:])
            krl = wp.tile([96, NT], F, tag="krl")
            nc.gpsimd.dma_start(krl[:], kflat[:].rearrange("c s -> (c s)").rearrange("(t p) -> p t", p=96))
            poi = wp.tile([96, NT], I32, tag="poi")
            nc.vector.tensor_copy(poi[:], rowrec[:, :, 6])
            keep_part = dp.tile([N, 1], F)
            zt = wp.tile([128, 64], F, tag="zt")
            nc.vector.memset(zt[:], 0.0)
            nc.gpsimd.dma_start(keep_part[:].rearrange("(p f) o -> p (f o)", p=128), zt[:])
            for t in range(NT):
                nc.gpsimd.indirect_dma_start(
                    out=keep_part[:],
                    out_offset=IOffs(ap=poi[:, t:t + 1], axis=0),
                    in_=krl[:, t:t + 1], in_offset=None,
                    bounds_check=N - 1, oob_is_err=False)
            keep_full = dp.tile([N, 1], F)
            nc.gpsimd.collective_compute(
                "AllReduce", Op.add, replica_groups=[list(range(NCORES))],
                ins=[keep_part[:]], outs=[keep_full[:]])

            # ---------- P5: global rank of scores_after ----------
            kf = wp.tile([128, 64], F, tag="kf")
            nc.gpsimd.dma_start(kf[:], keep_full[:].rearrange("(p f) o -> p (f o)", p=128))
            scf = wp.tile([128, 64], F, tag="scf")
            nc.gpsimd.dma_start(scf[:], scores_d[:].rearrange("(p f) -> p f", p=128))
            saf = wp.tile([128, 64], F, tag="saf")
            nc.vector.tensor_tensor(out=saf[:], in0=kf[:], in1=scf[:], op=Op.mult)
            sa_dram = dp.tile([N, 1], F)
            nc.gpsimd.dma_start(sa_dram[:].rearrange("(p f) o -> p (f o)", p=128), saf[:])
            if DEBUG:
                nc.gpsimd.dma_start(dbg["keep"][:].rearrange("(n o) -> n o", o=1), keep_full[:])
            _p5cm = tc.tile_pool(name="p5big", bufs=1)
            p5p = _p5cm.__enter__()
            sab = p5p.tile([128, N], F, tag="sab")
            nc.gpsimd.dma_start(sab[0:1, :], sa_dram[:].rearrange("n o -> o n"))
            pcount = 1
            while pcount < 128:
                nc.gpsimd.dma_start(sab[pcount:2 * pcount, :], sab[0:pcount, :])
                pcount *= 2
            crows = cp.tile([128, NCH], I32)
            nc.gpsimd.dma_start(crows[:], crows_d[:])
            s_row = wp.tile([128, NCH], F, tag="s_row")
            for u in range(NCH):
                nc.gpsimd.indirect_dma_start(
                    out=s_row[:, u:u + 1], out_offset=None,
                    in_=sa_dram[:],
                    in_offset=IOffs(ap=crows[:, u:u + 1], axis=0))
            r0 = wp.tile([128, NCH], F, tag="r0")
            eqa = wp.tile([128, NCH], F, tag="eqa")
            eqb = wp.tile([128, NCH], F, tag="eqb")
            nc.vector.memset(eqa[:], 0.0)
            junk5 = p5p.tile([128, N], F, tag="junk5")
            crowsf = wp.tile([128, NCH], F, tag="crowsf")
            nc.vector.tensor_copy(crowsf[:], crows[:])
            rpos = wp.tile([128, NCH], F, tag="rpos")
            for u in range(NCH):
                nc.vector.tensor_scalar(
                    out=rpos[:, u:u + 1], in0=crowsf[:, u:u + 1],
                    scalar1=float(1024 * u), scalar2=None, op0=Op.subtract)
            for u in range(NCH):
                nc.vector.tensor_scalar(
                    out=junk5[:], in0=sab[:], scalar1=s_row[:, u:u + 1], scalar2=0.0,
                    op0=Op.is_gt, op1=Op.add, accum_out=r0[:, u:u + 1])
                if u > 0:
                    nc.vector.tensor_scalar(
                        out=junk5[:, 0:1024 * u], in0=sab[:, 0:1024 * u],
                        scalar1=s_row[:, u:u + 1], scalar2=0.0,
                        op0=Op.is_equal, op1=Op.add, accum_out=eqa[:, u:u + 1])
                wm = wp.tile([128, 1024], F, tag="wm")
                nc.vector.tensor_scalar(out=wm[:], in0=io1024[:],
                                        scalar1=rpos[:, u:u + 1], scalar2=None,
                                        op0=Op.is_lt)
                nc.vector.scalar_tensor_tensor(
                    out=junk5[:, 0:1024], in0=sab[:, 1024 * u:1024 * (u + 1)],
                    scalar=s_row[:, u:u + 1], in1=wm[:],
                    op0=Op.is_equal, op1=Op.mult, accum_out=eqb[:, u:u + 1])
            rank = wp.tile([128, NCH], F, tag="rank")
            nc.vector.tensor_tensor(out=rank[:], in0=r0[:], in1=eqa[:], op=Op.add)
            nc.vector.tensor_tensor(out=rank[:], in0=rank[:], in1=eqb[:], op=Op.add)
            ranki = wp.tile([128, NCH], I32, tag="ranki")
            nc.vector.tensor_copy(ranki[:], rank[:])
            if DEBUG:
                nc.gpsimd.dma_start(dbg["rank"][:], rank[:])

            # ---------- P6: scatter final rows ----------
            obrow = wp.tile([128, NCH, 5], F, tag="obrow")
            for u in range(NCH):
                nc.gpsimd.indirect_dma_start(
                    out=obrow[:, u, 0:4], out_offset=None,
                    in_=boxes_d[:],
                    in_offset=IOffs(ap=crows[:, u:u + 1], axis=0))
            nc.vector.tensor_copy(obrow[:, :, 4], s_row[:])
            for u in range(NCH):
                nc.gpsimd.indirect_dma_start(
                    out=ob_d[:],
                    out_offset=IOffs(ap=ranki[:, u:u + 1], axis=0),
                    in_=obrow[:, u, :], in_offset=None)
                nc.gpsimd.indirect_dma_start(
                    out=inds_d[:].rearrange("(n o) -> n o", o=1),
                    out_offset=IOffs(ap=ranki[:, u:u + 1], axis=0),
                    in_=crows[:, u:u + 1], in_offset=None)
            _p5cm.__exit__(None, None, None)

    nc.finalize()
    return nc


def kernel(boxes, scores, idxs):
    from concourse.bass_utils import run_bass_kernel_spmd

    boxes = np.ascontiguousarray(np.asarray(boxes, dtype=np.float32))
    scores = np.ascontiguousarray(np.asarray(scores, dtype=np.float32))
    idxs_i = np.asarray(idxs).astype(np.int64)

    if "nc" not in _cache:
        _cache["nc"] = _build()
    nc = _cache["nc"]

    members = [np.where(idxs_i == c)[0].astype(np.int64) for c in range(C)]
    mx = max(len(m) for m in members)
    if mx > S:
        raise RuntimeError(f"class overflow: {mx} > {S}")

    in_maps = []
    for k in range(NCORES):
        sb = np.zeros((NSLOT, 4), np.float32)
        ss = np.full((NSLOT,), -1.0, np.float32)
        so = np.full((NSLOT,), float(N), np.float32)
        ctq = np.zeros((NT, 4), np.float32)
        for j in range(CPC):
            c = k * CPC + j
            m = members[c]
            n = len(m)
            sb[j * S:j * S + n] = boxes[m]
            ss[j * S:j * S + n] = scores[m]
            so[j * S:j * S + n] = m.astype(np.float32)
            ctq[2 * j:2 * j + 2, :] = float(c)
        p = np.arange(128)
        u = np.arange(NCH)
        crows = (1024 * u[None, :] + 8 * p[:, None] + k).astype(np.int32)
        in_maps.append({
            "boxes": boxes, "scores": scores,
            "slot_boxes": sb, "slot_scores": ss, "slot_origf": so,
            "cls_tq": ctq, "chunk_rows": crows,
        })

    trace = bool(int(os.environ.get("TRACE_NMS", "0")))
    if trace:
        try:
            import ntff_hook
            ntff_hook.install()
        except Exception as e:
            print("ntff hook install failed:", e)
        import tempfile
        kernel._tmpdir = tempfile.mkdtemp(prefix="nmsprof_")
        res = run_bass_kernel_spmd(nc, in_maps, list(range(NCORES)), trace=True,
                                   tmpdir=kernel._tmpdir)
    else:
        res = run_bass_kernel_spmd(nc, in_maps, list(range(NCORES)))
    outs = res.results
    kernel._res = res
    if DEBUG:
        kernel._last = (res, in_maps)
    ob = np.zeros((N, 5), np.float32)
    inds = np.zeros((N,), np.int64)
    for k in range(NCORES):
        ob += outs[k]["out_boxes"]
        inds += outs[k]["out_inds"].astype(np.int64)
    return ob, inds.astype(np.int32)
